# revision 26
# baseline (speedup 1.0000x reference)
"""Bass/Tile Trainium2 kernel for masked dot-product attention.

Problem: Q,K,V [2,16,2048,64] f32, attn_mask [2,1,2048,2048] bool (True = masked).
out = softmax(QK^T/8 masked) @ V, computed on 8 NeuronCores, batch*head sharded
(4 heads per core, each core's heads all in one batch so it needs one mask).

Device-side layout ("layout C" — transposed scores, row-tiled QK pairs, one
flat software-pipelined group stream):
  k-tiles t and t+8 form pair p=t; K^T for tile p sits in SBUF partitions
  0-63, tile p+8 in partitions 64-127 (Q^T is duplicated into both halves).
  Per group (pair p, q-chunk qc of 512), g = p*4 + qc:
    st2[128, 1024] f32 PSUM (2 banks):
      st2[:, 0:512]    = K_p  @ Q^T chunk   (TensorE tile_position (0,0))
      st2[:, 512:1024] = K_p8 @ Q^T chunk   (TensorE tile_position (64,0))
      -> the two 64-contraction matmuls run CONCURRENTLY in the PE array
         (row tiling), halving QK PE time vs sequential DK=64 matmuls.
    et2[128, 1024] bf16 = exp(st2 / 8)      (one ScalarE call, N=1024)
    et2 *= keep2[g]                         (one VectorE bf16 mul; keep = ~mask)
    ots[qc][65, 512] += Vp_p^T  @ et2[:, 0:512]    (TensorE accumulate;
    ots[qc][65, 512] += Vp_p8^T @ et2[:, 512:1024]  row 0 of vp is ones ->
                                                    row 0 of ots = denominators)
  All heads' groups form ONE flat stream; AV is lagged by `avlag` groups in
  that stream (crossing head boundaries), so the in-order PE queue never
  stalls on the exp/mask chain ahead of the next QK pair and there is no
  per-head AV flush burst. ScalarE (exp) is the bottleneck engine (~134us
  of the ~153us wall); everything else hides under it.
  Tail per head (12 pieces, spread over the next head's first 12 groups):
  DVE copy ev<-ots (frees the PSUM bank), reciprocal_approx_fast on the
  denominator row (partition 0 - the custom DVE op mishandles nonzero
  base partitions), gpsimd partition-broadcast, bf16 normalize multiply,
  DMA out in [d, q] bf16 (host transposes and casts to f32).
"""

import numpy as np
import ml_dtypes

B, H, S, DK = 2, 16, 2048, 64
NCORES = 8
HPC = (B * H) // NCORES  # heads per core = 4
KT = S // 128            # 16 k-tiles
NP = KT // 2             # 8 pairs
NG = NP * 4              # 32 groups (pair, q-chunk) per head
VPW = 72                 # v-block stride (1 ones + 64 v + pad to 16B align)
BF16 = ml_dtypes.bfloat16

_CACHE = {}


def _build(hpc=HPC, compile=True, reps=1,
           stages=("qk", "exp", "mask", "av", "tail"), avlag=8,
           iobufs=3, ebufs=10, stbufs=2, **_ignored):
    import contextlib
    import concourse.bass as bass
    import concourse.mybir as mybir
    import concourse.tile as tile
    from concourse import bacc
    HPC = hpc
    stages = set(stages)
    f32 = mybir.dt.float32
    bf16 = mybir.dt.bfloat16
    Exp = mybir.ActivationFunctionType.Exp

    nc = bacc.Bacc("TRN2", target_bir_lowering=False, debug=False,
                   num_devices=NCORES)

    qt_d = nc.dram_tensor("qt", [HPC, 128, S], bf16, kind="ExternalInput").ap()
    kt_d = nc.dram_tensor("kt", [HPC, 128, NP * 128], bf16,
                          kind="ExternalInput").ap()
    vp_d = nc.dram_tensor("vp", [HPC, 128, KT * VPW], bf16,
                          kind="ExternalInput").ap()
    mk_d = nc.dram_tensor("maskt", [128, NG * 1024], bf16,
                          kind="ExternalInput").ap()
    out_d = nc.dram_tensor("out", [HPC, DK, S], bf16,
                           kind="ExternalOutput").ap()

    with tile.TileContext(nc) as tc:
        with (
            tc.tile_pool(name="const", bufs=1) as const,
            tc.tile_pool(name="io", bufs=iobufs) as io,
            tc.tile_pool(name="epool", bufs=ebufs) as epool,
            tc.tile_pool(name="fin", bufs=4) as fin,
            tc.tile_pool(name="ps_s", bufs=stbufs, space="PSUM") as ps_s,
            tc.tile_pool(name="ps_o", bufs=4, space="PSUM") as ps_o,
        ):
            mask_sb = const.tile([128, NG * 1024], bf16)
            mask_v = mask_sb.rearrange("p (g q) -> p g q", g=NG)

            loop_ctx = (tc.For_i(0, reps, 1) if reps > 1
                        else contextlib.nullcontext())

            state = {}
            NTAIL = 12

            def head_setup(h):
                qt_sb = io.tile([128, S], bf16, tag="qt")
                nc.sync.dma_start(out=qt_sb[:, :1024], in_=qt_d[h][:, :1024])
                kt_sb = io.tile([128, NP * 128], bf16, tag="kt")
                nc.sync.dma_start(out=kt_sb, in_=kt_d[h])
                nc.sync.dma_start(out=qt_sb[:, 1024:], in_=qt_d[h][:, 1024:])
                vp_sb = io.tile([128, KT * VPW], bf16, tag="vp")
                nc.sync.dma_start(out=vp_sb, in_=vp_d[h])
                if h == 0:
                    for i in range(8):
                        nc.sync.dma_start(
                            out=mask_sb[:, i * 4096:(i + 1) * 4096],
                            in_=mk_d[:, i * 4096:(i + 1) * 4096])
                ots = []
                for qc in range(4):
                    ot = ps_o.tile([65, 512], f32, tag="ot",
                                   name=f"ot_h{h}_q{qc}")
                    ots.append(ot)
                ost = io.tile([65, S], bf16, tag="ost", name=f"ost_h{h}")
                state[h] = {"ots": ots, "ost": ost, "ev": {}, "bc": {},
                            "et": [None] * ebufs,
                            "qt": qt_sb, "kt": kt_sb,
                            "vp": vp_sb.rearrange("p (k c) -> p k c", k=KT)}

            def emit_av(h, g):
                st = state[h]
                p, qc = divmod(g, 4)
                ot = st["ots"][qc]
                et = st["et"][g % ebufs]
                nc.tensor.matmul(ot, st["vp"][:, p, :65], et[:, 0:512],
                                 start=(p == 0), stop=False)
                nc.tensor.matmul(ot, st["vp"][:, p + NP, :65],
                                 et[:, 512:1024],
                                 start=False, stop=(p == NP - 1))

            def emit_tail_piece(h, step):
                # row 0 of ots is the denominator (ones column first in vp).
                # step 0-3: evacuate ots -> ev (frees PSUM for next head)
                # step 4-7: reciprocal of denominator row + gpsimd broadcast
                # step 8-11: final multiply + store chunk
                st = state[h]
                qc = step % 4
                sl = slice(qc * 512, (qc + 1) * 512)
                if step < 4:
                    ev = fin.tile([65, 512], f32, tag="ev", bufs=8,
                                  name=f"ev_h{h}_q{qc}")
                    nc.vector.tensor_copy(ev, st["ots"][qc])
                    st["ev"][qc] = ev
                elif step < 8:
                    rc1 = fin.tile([1, 512], f32, tag="rc1", bufs=8,
                                   name=f"rc1_h{h}_q{qc}")
                    nc.vector.reciprocal_approx_fast(rc1,
                                                     st["ev"][qc][0:1, :])
                    bc = fin.tile([65, 512], f32, tag="bc", bufs=4,
                                  name=f"bc_h{h}_q{qc}")
                    nc.gpsimd.partition_broadcast(bc, rc1)
                    st["bc"][qc] = bc
                else:
                    nc.vector.tensor_mul(st["ost"][:, sl], st["ev"][qc],
                                         st["bc"][qc])
                    nc.gpsimd.dma_start(out=out_d[h][:, sl],
                                        in_=st["ost"][1:65, sl])

            NGG = HPC * NG

            with loop_ctx:
                for gg in range(NGG):
                    if True:
                        h, g = divmod(gg, NG)
                        if g == 0:
                            head_setup(h)
                        p, qc = divmod(g, 4)
                        st2 = ps_s.tile([128, 1024], f32, tag="st",
                                        name=f"st_h{h}_g{g}")
                        if "qk" in stages:
                            nc.tensor.matmul(
                                st2[:, 0:512],
                                state[h]["kt"][0:64, p * 128:(p + 1) * 128],
                                state[h]["qt"][0:64, qc * 512:(qc + 1) * 512],
                                start=True, stop=True, tile_position=(0, 0))
                            nc.tensor.matmul(
                                st2[:, 512:1024],
                                state[h]["kt"][64:128, p * 128:(p + 1) * 128],
                                state[h]["qt"][64:128,
                                               qc * 512:(qc + 1) * 512],
                                start=True, stop=True, tile_position=(64, 0))
                        et2 = epool.tile([128, 1024], bf16, tag="et",
                                         name=f"et_h{h}_g{g}")
                        state[h]["et"][g % ebufs] = et2
                        if "exp" in stages:
                            nc.scalar.activation(et2, st2, Exp,
                                                 scale=1.0 / np.sqrt(DK))
                        if "mask" in stages:
                            nc.vector.tensor_mul(et2, et2, mask_v[:, g, :])
                    if "av" in stages and gg >= avlag:
                        g2 = gg - avlag
                        emit_av(g2 // NG, g2 % NG)
                    h, g = divmod(gg, NG)
                    toff = max(avlag - 4, 0)
                    if (h > 0 and toff <= g < toff + NTAIL
                            and "tail" in stages):
                        emit_tail_piece(h - 1, g - toff)

                # epilogue: flush last AVs, then the last head's tail
                if "av" in stages:
                    for g2 in range(NGG - avlag, NGG):
                        emit_av(g2 // NG, g2 % NG)
                if "tail" in stages:
                    for step in range(NTAIL):
                        emit_tail_piece(HPC - 1, step)

    if compile:
        nc.compile()
    return nc


def _get_nc():
    if "nc" not in _CACHE:
        _CACHE["nc"] = _build()
    return _CACHE["nc"]


def _shard(Q, K, V, attn_mask):
    """Host-side marshalling: shard/transposes per core."""
    Q = np.asarray(Q, np.float32)
    K = np.asarray(K, np.float32)
    V = np.asarray(V, np.float32)
    attn_mask = np.asarray(attn_mask, bool)

    # keep2[b][128, g=(p,qc), 1024] = [keepT(tile p) | keepT(tile p+8)] for
    # q columns qc*512:(qc+1)*512, where keepT[b, kp, t, q] = ~mask[b, q, k]
    keep = (~attn_mask[:, 0]).astype(BF16)                   # [B, q, k]
    mkT = keep.transpose(0, 2, 1)                            # [B, k, q]
    mkT = mkT.reshape(B, KT, 128, S)                         # [B, t, kp, q]
    m2 = np.empty((B, 128, NP, 4, 2, 512), BF16)
    for p in range(NP):
        for qc in range(4):
            m2[:, :, p, qc, 0, :] = mkT[:, p, :, qc * 512:(qc + 1) * 512]
            m2[:, :, p, qc, 1, :] = mkT[:, p + NP, :,
                                        qc * 512:(qc + 1) * 512]
    m2 = np.ascontiguousarray(m2).reshape(B, 128, NG * 1024)

    in_maps = []
    for c in range(NCORES):
        b = c // (NCORES // B)
        h0 = (c % (NCORES // B)) * HPC
        QT = np.ascontiguousarray(
            Q[b, h0:h0 + HPC].transpose(0, 2, 1)).astype(BF16)  # [HPC, DK, S]
        QT2 = np.concatenate([QT, QT], axis=1)                  # [HPC, 128, S]
        KTt = np.ascontiguousarray(
            K[b, h0:h0 + HPC].transpose(0, 2, 1)).astype(BF16)  # [HPC, DK, S]
        # kt[h, 0:64, p*128:...] = tile p; kt[h, 64:128, ...] = tile p+8
        kt2 = np.empty((HPC, 128, NP * 128), BF16)
        kt2[:, 0:64, :] = KTt[:, :, :NP * 128]
        kt2[:, 64:128, :] = KTt[:, :, NP * 128:]
        vp = np.zeros((HPC, 128, KT, VPW), BF16)
        vp[:, :, :, 0] = 1.0
        vp[:, :, :, 1:DK + 1] = V[b, h0:h0 + HPC].astype(BF16).reshape(
            HPC, KT, 128, DK).transpose(0, 2, 1, 3)
        in_maps.append({
            "qt": QT2,
            "kt": kt2,
            "vp": np.ascontiguousarray(vp).reshape(HPC, 128, KT * VPW),
            "maskt": m2[b],
        })
    return in_maps


def kernel(Q, K, V, attn_mask):
    from concourse.bass_utils import run_bass_kernel_spmd

    nc = _get_nc()
    in_maps = _shard(Q, K, V, attn_mask)
    res = run_bass_kernel_spmd(nc, in_maps, list(range(NCORES)))
    out = np.empty((B, H, S, DK), np.float32)
    for c in range(NCORES):
        b = c // (NCORES // B)
        h0 = (c % (NCORES // B)) * HPC
        out[b, h0:h0 + HPC] = res.results[c]["out"].astype(
            np.float32).transpose(0, 2, 1)
    return out
